# revision 15
# baseline (speedup 1.0000x reference)
"""Trainium2 Bass kernel for nn_AffineAttentionNN (moe_routing).

Math (per the reference):
    dist_sq[n,c] = ||x[n]-ctrs[c]||^2_s   (s-weighted squared distance)
    a = softmax(-dist_sq, axis=c)
    out = einsum('nc,ng,cgp->np', a, x, Wv) + a @ Ov

Device decomposition (data-parallel over n across 8 cores; per core n_loc=2048):
  - Softmax offsets: -dist_sq = -(x*x)@s + 2(x*s)@ctrs.T - (ctrs*ctrs)@s.
    The per-row term (x*x)@s is constant along c, so it cancels in softmax;
    we exponentiate g[c,n] = 2(x*s)@ctrs.T - ccs[c] directly (args are all <= 0
    for this data regime; underflow hits only weights ~1e-20 relative).
  - All tensors live transposed ([feature, n] layouts) so the TensorE
    contraction dim is on partitions and the host does the final .T:
      gT = matmul(lhsT=2*s*ctrs (g,c layout), rhs=xT)          [c, n] (PSUM)
      eT = Exp(gT + bias=-ccs[c])  on ScalarE                  [c, n]
      Z  = matmul(lhsT=ones, rhs=eT) -> reciprocal             [1, n]
      per expert c: eRep = DMA-broadcast of eT[c,:] to 128 partitions
                    zT   = xT * eRep   (VectorE)               [g, n]
                    outT[p, jslice] += matmul(lhsT=Wv[c] (g,p), rhs=zT)  PSUM acc
      outT += matmul(lhsT=Ov (c,p), rhs=eT)                    (Ov term)
      out  = outT * recipZ_rep  (normalize), DMA out, host transposes.
"""

import os
import numpy as np
from contextlib import ExitStack

import concourse.bass as bass
import concourse.tile as tile
from concourse import mybir

N, D, C, P = 16384, 128, 128, 128
N_CORES = 8
N_LOC = N // N_CORES          # 2048
CHUNK = 512                   # PSUM bank width (fp32)
NCH = N_LOC // CHUNK          # 4

F32 = mybir.dt.float32
F32R = mybir.dt.float32r

# MM_MODE: "f32" (exact, 4 cyc/row) or "f32r" (1 cyc/row at N>=256, TF32-ish)
MM_MODE = os.environ.get("KERNEL_MM_MODE", "f32r")
# Fraction of experts whose eRep is built by TensorE rank-1 matmuls into PSUM
# (instead of a 1 MB DMA broadcast): experts with c % DEN < NUM take the PE path.
if MM_MODE == "f32r":
    _rep = os.environ.get("KERNEL_REP", "2/3").split("/")
    REP_NUM, REP_DEN = int(_rep[0]), int(_rep[1])
else:
    REP_NUM, REP_DEN = 0, 1
REP_HALF = N_LOC // 2  # PE-replication PSUM tile width (2 banks)


def _legalize_waits(nc, max_waits=1):
    """This walrus build accepts at most one sync-wait per instruction; Tile
    emits several. Hoist the excess onto standalone single-wait
    InstEventSemaphore ops just before the owner on the same engine stream."""
    import bass_rust

    n = 0
    for f in nc.m.functions:
        for blk in f.blocks:
            out = []
            for inst in blk.instructions:
                si = getattr(inst, "sync_info", None)
                waits = list(si.on_wait) if si is not None else []
                if len(waits) > max_waits:
                    extra, keep = waits[:-max_waits], waits[-max_waits:]
                    for w in extra:
                        n += 1
                        ev = mybir.InstEventSemaphore(
                            name=f"legal_wait_{n}_{inst.name}", ins=[], outs=[]
                        )
                        ev.engine = inst.engine
                        ev.sync_info = bass_rust.SyncInfo(on_wait=[w], on_update=[])
                        out.append(ev)
                    inst.sync_info = bass_rust.SyncInfo(
                        on_wait=keep, on_update=list(si.on_update)
                    )
                out.append(inst)
            blk.instructions = out
    return n


def _emit_kernel(tc, aps):
    nc = tc.nc
    xT, wvT, c2sT, nccs, ov, outT = (
        aps["xT"], aps["wvT"], aps["c2sT"], aps["nccs"], aps["ov"], aps["outT"],
    )
    mm_dt = F32R if MM_MODE == "f32r" else F32

    with ExitStack() as ctx:
        const = ctx.enter_context(tc.tile_pool(name="const", bufs=1))
        dram = ctx.enter_context(tc.tile_pool(name="dram", bufs=1, space="DRAM"))
        erep_p = ctx.enter_context(tc.tile_pool(name="erep", bufs=3))
        stage_p = ctx.enter_context(tc.tile_pool(name="stage", bufs=3))
        z_p = ctx.enter_context(tc.tile_pool(name="zt", bufs=3))
        out_p = ctx.enter_context(tc.tile_pool(name="outs", bufs=1))

        # ---- constants / inputs into SBUF ----
        mm_sb_dt = F32R if mm_dt == F32R else F32
        use_rep = REP_NUM > 0
        xT_s = const.tile([D, N_LOC], F32, tag="xT")
        nc.sync.dma_start(xT_s[:], xT[:, :])
        wv_s = const.tile([D, C * P], mm_sb_dt, tag="wv")
        for k in range(8):
            w = C * P // 8
            nc.sync.dma_start(wv_s[:, k * w:(k + 1) * w], wvT[:, k * w:(k + 1) * w])
        c2s_s = const.tile([D, C], F32, tag="c2s")
        nc.sync.dma_start(c2s_s[:], c2sT[:, :])
        nccs_s = const.tile([C, 1], F32, tag="nccs")
        nc.sync.dma_start(nccs_s[:], nccs[:, :])
        ov_s = const.tile([C, P], F32, tag="ov")
        nc.sync.dma_start(ov_s[:], ov[:, :])
        ones_s = const.tile([C, 1], F32, tag="ones")
        nc.vector.memset(ones_s[:], 1.0)
        eT_s = const.tile([C, N_LOC], F32, tag="eT")
        rz_s = const.tile([1, N_LOC], F32, tag="rz")
        rzrep_s = const.tile([P, N_LOC], F32, tag="rzrep")
        if use_rep:
            ones_r = const.tile([1, P], F32R, tag="ones_r")
            nc.sync.dma_start(ones_r[:], aps["onesr"][0:1, :])

        e_dram = dram.tile([C, N_LOC], F32, tag="e_dram")
        rz_dram = dram.tile([1, N_LOC], F32, tag="rz_dram")

        # ---- prologue: distances -> unnormalized softmax weights eT [c, n] ----
        with ExitStack() as dctx:
            psum_d = dctx.enter_context(
                tc.tile_pool(name="psum_d", bufs=2, space="PSUM"))
            psum_z = dctx.enter_context(
                tc.tile_pool(name="psum_z", bufs=1, space="PSUM"))
            for j in range(NCH):
                js = slice(j * CHUNK, (j + 1) * CHUNK)
                pd = psum_d.tile([C, CHUNK], F32, tag="pd")
                nc.tensor.matmul(pd[:], c2s_s[:], xT_s[:, js], start=True, stop=True)
                nc.scalar.activation(
                    eT_s[:, js], pd[:], mybir.ActivationFunctionType.Exp,
                    bias=nccs_s[:, 0:1], scale=1.0,
                )
                pz = psum_z.tile([1, CHUNK], F32, tag="pz")
                nc.tensor.matmul(pz[:], ones_s[:], eT_s[:, js], start=True, stop=True)
                nc.vector.reciprocal(rz_s[0:1, js], pz[0:1, :])

        # ---- roundtrip e / recipZ through DRAM for partition-broadcast ----
        nc.sync.dma_start(e_dram[:, :], eT_s[:])
        nc.sync.dma_start(rz_dram[:, :], rz_s[:])
        nc.sync.dma_start(rzrep_s[:], rz_dram[0:1, :].partition_broadcast(P))

        # ---- main expert loop, accumulate outT in PSUM ----
        psum_o = ctx.enter_context(tc.tile_pool(name="psum_o", bufs=1, space="PSUM"))
        if use_rep:
            psum_r = ctx.enter_context(
                tc.tile_pool(name="psum_r", bufs=2, space="PSUM"))
        po = psum_o.tile([P, N_LOC], F32, tag="po")
        for c in range(C):
            wv_c = wv_s[:, c * P:(c + 1) * P]
            if use_rep and (c % REP_DEN) < REP_NUM:
                # Stage the 8 KB e-row at partition 0, then TensorE rank-1
                # replication into PSUM; DVE multiplies from PSUM. Avoids the
                # 1 MB broadcast DMA per expert.
                stg = stage_p.tile([1, N_LOC], F32R, tag="stg")
                nc.sync.dma_start(stg[:], e_dram[c:c + 1, :].bitcast(F32R))
                for h in range(2):
                    hs = slice(h * REP_HALF, (h + 1) * REP_HALF)
                    pr = psum_r.tile([D, REP_HALF], F32, tag="pr")
                    for q in range(REP_HALF // CHUNK):
                        qs_dst = slice(q * CHUNK, (q + 1) * CHUNK)
                        qs_src = slice(h * REP_HALF + q * CHUNK,
                                       h * REP_HALF + (q + 1) * CHUNK)
                        nc.tensor.matmul(
                            pr[:, qs_dst], ones_r[:], stg[0:1, qs_src],
                            start=True, stop=True,
                        )
                    z = z_p.tile([D, REP_HALF], mm_sb_dt, tag="z")
                    nc.vector.tensor_mul(z[:], xT_s[:, hs], pr[:])
                    for q in range(REP_HALF // CHUNK):
                        qs = slice(q * CHUNK, (q + 1) * CHUNK)
                        jfull = slice(h * REP_HALF + q * CHUNK,
                                      h * REP_HALF + (q + 1) * CHUNK)
                        nc.tensor.matmul(
                            po[:, jfull], wv_c, z[:, qs],
                            start=(c == 0), stop=False, skip_group_check=True,
                        )
            else:
                er = erep_p.tile([D, N_LOC], F32, tag="er")
                nc.sync.dma_start(er[:], e_dram[c:c + 1, :].partition_broadcast(D))
                z = z_p.tile([D, N_LOC], mm_sb_dt, tag="zf")
                nc.vector.tensor_mul(z[:], xT_s[:], er[:])
                for j in range(NCH):
                    js = slice(j * CHUNK, (j + 1) * CHUNK)
                    nc.tensor.matmul(
                        po[:, js], wv_c, z[:, js],
                        start=(c == 0), stop=False, skip_group_check=True,
                    )

        # ---- Ov term (exact f32), closes the accumulation groups ----
        for j in range(NCH):
            js = slice(j * CHUNK, (j + 1) * CHUNK)
            nc.tensor.matmul(
                po[:, js], ov_s[:], eT_s[:, js],
                start=False, stop=True, skip_group_check=True,
            )

        # ---- normalize and store ----
        out_s = out_p.tile([P, N_LOC], F32, tag="out")
        for j in range(NCH):
            js = slice(j * CHUNK, (j + 1) * CHUNK)
            nc.vector.tensor_mul(out_s[:, js], po[:, js], rzrep_s[:, js])
        nc.sync.dma_start(outT[:, :], out_s[:])


def build_nc():
    nc = bass.Bass(target_bir_lowering=False, trn_type="TRN2")
    wv_dt = F32R if MM_MODE == "f32r" else F32
    aps = {
        "xT": nc.dram_tensor("xT", [D, N_LOC], F32, kind="ExternalInput").ap(),
        "wvT": nc.dram_tensor("wvT", [D, C * P], wv_dt, kind="ExternalInput").ap(),
        "c2sT": nc.dram_tensor("c2sT", [D, C], F32, kind="ExternalInput").ap(),
        "nccs": nc.dram_tensor("nccs", [C, 1], F32, kind="ExternalInput").ap(),
        "ov": nc.dram_tensor("ov", [C, P], F32, kind="ExternalInput").ap(),
        "outT": nc.dram_tensor("outT", [P, N_LOC], F32, kind="ExternalOutput").ap(),
    }
    if REP_NUM > 0:
        aps["onesr"] = nc.dram_tensor(
            "onesr", [1, P], F32R, kind="ExternalInput").ap()
    with tile.TileContext(nc) as tc:
        _emit_kernel(tc, aps)
    _legalize_waits(nc)
    return nc


_CACHE = {}


def _get_nc():
    key = MM_MODE
    if key not in _CACHE:
        _CACHE[key] = build_nc()
    return _CACHE[key]


def kernel(x, ctrs, Wv, Ov, s, _spmd_kwargs=None):
    from concourse.bass_utils import run_bass_kernel_spmd

    x = np.ascontiguousarray(x, dtype=np.float32)
    ctrs = np.asarray(ctrs, dtype=np.float32)
    Wv = np.asarray(Wv, dtype=np.float32)
    Ov = np.ascontiguousarray(Ov, dtype=np.float32)
    s = np.asarray(s, dtype=np.float32)

    # host-side prep of the small routing constants
    c2sT = np.ascontiguousarray((2.0 * ctrs * s[None, :]).T)        # [g, c]
    nccs = np.ascontiguousarray(-((ctrs * ctrs) @ s)[:, None])      # [c, 1]
    wvT = np.ascontiguousarray(Wv.transpose(1, 0, 2).reshape(D, C * P))  # [g, c*p]

    in_maps = []
    for i in range(N_CORES):
        xi = x[i * N_LOC:(i + 1) * N_LOC]
        m = {
            "xT": np.ascontiguousarray(xi.T),
            "wvT": wvT,
            "c2sT": c2sT,
            "nccs": nccs,
            "ov": Ov,
        }
        if REP_NUM > 0:
            m["onesr"] = np.ones((1, P), dtype=np.float32)
        in_maps.append(m)

    nc = _get_nc()
    res = run_bass_kernel_spmd(
        nc, in_maps, core_ids=list(range(N_CORES)), **(_spmd_kwargs or {})
    )
    out = np.empty((N, P), dtype=np.float32)
    for i in range(N_CORES):
        out[i * N_LOC:(i + 1) * N_LOC] = res.results[i]["outT"].T
    kernel.last_result = res
    return out


# revision 18
# speedup vs baseline: 1.8272x; 1.8272x over previous
"""Trainium2 Bass kernel for nn_AffineAttentionNN (moe_routing).

Math (per the reference):
    dist_sq[n,c] = ||x[n]-ctrs[c]||^2_s   (s-weighted squared distance)
    a = softmax(-dist_sq, axis=c)
    out = einsum('nc,ng,cgp->np', a, x, Wv) + a @ Ov

Device decomposition (data-parallel over n across 8 cores; per core n_loc=2048):
  - Softmax offsets: -dist_sq = -(x*x)@s + 2(x*s)@ctrs.T - (ctrs*ctrs)@s.
    The per-row term (x*x)@s is constant along c, so it cancels in softmax;
    we exponentiate g[c,n] = 2(x*s)@ctrs.T - ccs[c] directly (args are all <= 0
    for this data regime; underflow hits only weights ~1e-20 relative).
  - All tensors live transposed ([feature, n] layouts) so the TensorE
    contraction dim is on partitions and the host does the final .T:
      gT = matmul(lhsT=2*s*ctrs (g,c layout), rhs=xT)          [c, n] (PSUM)
      eT = Exp(gT + bias=-ccs[c])  on ScalarE                  [c, n]
      Z  = matmul(lhsT=ones, rhs=eT) -> reciprocal             [1, n]
      per expert c: eRep = DMA-broadcast of eT[c,:] to 128 partitions
                    zT   = xT * eRep   (VectorE)               [g, n]
                    outT[p, jslice] += matmul(lhsT=Wv[c] (g,p), rhs=zT)  PSUM acc
      outT += matmul(lhsT=Ov (c,p), rhs=eT)                    (Ov term)
      out  = outT * recipZ_rep  (normalize), DMA out, host transposes.
"""

import os
import numpy as np
from contextlib import ExitStack

import concourse.bass as bass
import concourse.tile as tile
from concourse import mybir

N, D, C, P = 16384, 128, 128, 128
N_CORES = 8
N_LOC = N // N_CORES          # 2048
CHUNK = 512                   # PSUM bank width (fp32)
NCH = N_LOC // CHUNK          # 4

F32 = mybir.dt.float32
F32R = mybir.dt.float32r

# MM_MODE: "f32" (exact, 4 cyc/row) or "f32r" (1 cyc/row at N>=256, TF32-ish)
MM_MODE = os.environ.get("KERNEL_MM_MODE", "f32r")
# Fraction of experts whose eRep is built by TensorE rank-1 matmuls into PSUM
# (instead of a 1 MB DMA broadcast): experts with c % DEN < NUM take the PE path.
if MM_MODE == "f32r":
    _rep = os.environ.get("KERNEL_REP", "0/1").split("/")
    REP_NUM, REP_DEN = int(_rep[0]), int(_rep[1])
else:
    REP_NUM, REP_DEN = 0, 1
REP_HALF = N_LOC // 2  # PE-replication PSUM tile width (2 banks)


def _legalize_waits(nc, max_waits=1):
    """This walrus build accepts at most one sync-wait per instruction; Tile
    emits several. Hoist the excess onto standalone single-wait
    InstEventSemaphore ops just before the owner on the same engine stream."""
    import bass_rust

    n = 0
    for f in nc.m.functions:
        for blk in f.blocks:
            out = []
            for inst in blk.instructions:
                si = getattr(inst, "sync_info", None)
                waits = list(si.on_wait) if si is not None else []
                if len(waits) > max_waits:
                    extra, keep = waits[:-max_waits], waits[-max_waits:]
                    for w in extra:
                        n += 1
                        ev = mybir.InstEventSemaphore(
                            name=f"legal_wait_{n}_{inst.name}", ins=[], outs=[]
                        )
                        ev.engine = inst.engine
                        ev.sync_info = bass_rust.SyncInfo(on_wait=[w], on_update=[])
                        out.append(ev)
                    inst.sync_info = bass_rust.SyncInfo(
                        on_wait=keep, on_update=list(si.on_update)
                    )
                out.append(inst)
            blk.instructions = out
    return n


def _emit_kernel(tc, aps):
    nc = tc.nc
    xT, wvT, c2sT, nccs, ov, outT = (
        aps["xT"], aps["wvT"], aps["c2sT"], aps["nccs"], aps["ov"], aps["outT"],
    )
    mm_dt = F32R if MM_MODE == "f32r" else F32

    with ExitStack() as ctx:
        const = ctx.enter_context(tc.tile_pool(name="const", bufs=1))
        dram = ctx.enter_context(tc.tile_pool(name="dram", bufs=1, space="DRAM"))
        erep_p = ctx.enter_context(tc.tile_pool(name="erep", bufs=5))
        stage_p = ctx.enter_context(tc.tile_pool(name="stage", bufs=3))
        z_p = ctx.enter_context(tc.tile_pool(name="zt", bufs=4))
        out_p = ctx.enter_context(tc.tile_pool(name="outs", bufs=1))

        # ---- constants / inputs into SBUF ----
        mm_sb_dt = F32R if mm_dt == F32R else F32
        use_rep = REP_NUM > 0
        xT_s = const.tile([D, N_LOC], F32, tag="xT")
        nc.sync.dma_start(xT_s[:], xT[:, :])
        wv_s = const.tile([D, C * P], mm_sb_dt, tag="wv")
        for k in range(8):
            w = C * P // 8
            nc.sync.dma_start(wv_s[:, k * w:(k + 1) * w], wvT[:, k * w:(k + 1) * w])
        c2s_s = const.tile([D, C], F32, tag="c2s")
        nc.sync.dma_start(c2s_s[:], c2sT[:, :])
        nccs_s = const.tile([C, 1], F32, tag="nccs")
        nc.sync.dma_start(nccs_s[:], nccs[:, :])
        ov_s = const.tile([C, P], F32, tag="ov")
        nc.sync.dma_start(ov_s[:], ov[:, :])
        ones_s = const.tile([C, 1], F32, tag="ones")
        nc.vector.memset(ones_s[:], 1.0)
        eT_s = const.tile([C, N_LOC], F32, tag="eT")
        rz_s = const.tile([1, N_LOC], F32, tag="rz")
        rzrep_s = const.tile([P, N_LOC], F32, tag="rzrep")
        if use_rep:
            ones_r = const.tile([1, P], F32R, tag="ones_r")
            nc.sync.dma_start(ones_r[:], aps["onesr"][0:1, :])

        e_dram = dram.tile([C, N_LOC], F32, tag="e_dram")
        rz_dram = dram.tile([1, N_LOC], F32, tag="rz_dram")

        # ---- prologue: distances -> unnormalized softmax weights eT [c, n] ----
        with ExitStack() as dctx:
            psum_d = dctx.enter_context(
                tc.tile_pool(name="psum_d", bufs=2, space="PSUM"))
            psum_z = dctx.enter_context(
                tc.tile_pool(name="psum_z", bufs=1, space="PSUM"))
            for j in range(NCH):
                js = slice(j * CHUNK, (j + 1) * CHUNK)
                pd = psum_d.tile([C, CHUNK], F32, tag="pd")
                nc.tensor.matmul(pd[:], c2s_s[:], xT_s[:, js], start=True, stop=True)
                nc.scalar.activation(
                    eT_s[:, js], pd[:], mybir.ActivationFunctionType.Exp,
                    bias=nccs_s[:, 0:1], scale=1.0,
                )
                pz = psum_z.tile([1, CHUNK], F32, tag="pz")
                nc.tensor.matmul(pz[:], ones_s[:], eT_s[:, js], start=True, stop=True)
                nc.vector.reciprocal(rz_s[0:1, js], pz[0:1, :])

        # ---- roundtrip e / recipZ through DRAM for partition-broadcast ----
        nc.sync.dma_start(e_dram[:, :], eT_s[:])
        nc.sync.dma_start(rz_dram[:, :], rz_s[:])
        nc.sync.dma_start(rzrep_s[:], rz_dram[0:1, :].partition_broadcast(P))

        # ---- main expert loop, accumulate outT in PSUM ----
        psum_o = ctx.enter_context(tc.tile_pool(name="psum_o", bufs=1, space="PSUM"))
        if use_rep:
            psum_r = ctx.enter_context(
                tc.tile_pool(name="psum_r", bufs=2, space="PSUM"))
        po = psum_o.tile([P, N_LOC], F32, tag="po")
        for c in range(C):
            wv_c = wv_s[:, c * P:(c + 1) * P]
            if use_rep and (c % REP_DEN) < REP_NUM:
                # Stage the 8 KB e-row at partition 0, then TensorE rank-1
                # replication into PSUM; DVE multiplies from PSUM. Avoids the
                # 1 MB broadcast DMA per expert.
                stg = stage_p.tile([1, N_LOC], F32R, tag="stg")
                nc.sync.dma_start(stg[:], e_dram[c:c + 1, :].bitcast(F32R))
                for h in range(2):
                    hs = slice(h * REP_HALF, (h + 1) * REP_HALF)
                    pr = psum_r.tile([D, REP_HALF], F32, tag="pr")
                    for q in range(REP_HALF // CHUNK):
                        qs_dst = slice(q * CHUNK, (q + 1) * CHUNK)
                        qs_src = slice(h * REP_HALF + q * CHUNK,
                                       h * REP_HALF + (q + 1) * CHUNK)
                        nc.tensor.matmul(
                            pr[:, qs_dst], ones_r[:], stg[0:1, qs_src],
                            start=True, stop=True,
                        )
                    z = z_p.tile([D, REP_HALF], mm_sb_dt, tag="z")
                    nc.vector.tensor_mul(z[:], xT_s[:, hs], pr[:])
                    for q in range(REP_HALF // CHUNK):
                        qs = slice(q * CHUNK, (q + 1) * CHUNK)
                        jfull = slice(h * REP_HALF + q * CHUNK,
                                      h * REP_HALF + (q + 1) * CHUNK)
                        nc.tensor.matmul(
                            po[:, jfull], wv_c, z[:, qs],
                            start=(c == 0), stop=False, skip_group_check=True,
                        )
            else:
                er = erep_p.tile([D, N_LOC], F32, tag="er")
                # two half-width DMAs land on different HWDGE queues -> lower
                # per-expert broadcast latency
                for h in range(2):
                    hs = slice(h * REP_HALF, (h + 1) * REP_HALF)
                    nc.sync.dma_start(
                        er[:, hs], e_dram[c:c + 1, hs].partition_broadcast(D))
                z = z_p.tile([D, N_LOC], mm_sb_dt, tag="zf")
                nc.vector.tensor_mul(z[:], xT_s[:], er[:])
                for j in range(NCH):
                    js = slice(j * CHUNK, (j + 1) * CHUNK)
                    nc.tensor.matmul(
                        po[:, js], wv_c, z[:, js],
                        start=(c == 0), stop=False, skip_group_check=True,
                    )

        # ---- Ov term (exact f32), closes the accumulation groups ----
        for j in range(NCH):
            js = slice(j * CHUNK, (j + 1) * CHUNK)
            nc.tensor.matmul(
                po[:, js], ov_s[:], eT_s[:, js],
                start=False, stop=True, skip_group_check=True,
            )

        # ---- normalize and store ----
        out_s = out_p.tile([P, N_LOC], F32, tag="out")
        for j in range(NCH):
            js = slice(j * CHUNK, (j + 1) * CHUNK)
            nc.vector.tensor_mul(out_s[:, js], po[:, js], rzrep_s[:, js])
        nc.sync.dma_start(outT[:, :], out_s[:])


def build_nc():
    nc = bass.Bass(target_bir_lowering=False, trn_type="TRN2")
    wv_dt = F32R if MM_MODE == "f32r" else F32
    aps = {
        "xT": nc.dram_tensor("xT", [D, N_LOC], F32, kind="ExternalInput").ap(),
        "wvT": nc.dram_tensor("wvT", [D, C * P], wv_dt, kind="ExternalInput").ap(),
        "c2sT": nc.dram_tensor("c2sT", [D, C], F32, kind="ExternalInput").ap(),
        "nccs": nc.dram_tensor("nccs", [C, 1], F32, kind="ExternalInput").ap(),
        "ov": nc.dram_tensor("ov", [C, P], F32, kind="ExternalInput").ap(),
        "outT": nc.dram_tensor("outT", [P, N_LOC], F32, kind="ExternalOutput").ap(),
    }
    if REP_NUM > 0:
        aps["onesr"] = nc.dram_tensor(
            "onesr", [1, P], F32R, kind="ExternalInput").ap()
    with tile.TileContext(nc) as tc:
        _emit_kernel(tc, aps)
    _legalize_waits(nc)
    return nc


_CACHE = {}


def _get_nc():
    key = MM_MODE
    if key not in _CACHE:
        _CACHE[key] = build_nc()
    return _CACHE[key]


def kernel(x, ctrs, Wv, Ov, s, _spmd_kwargs=None):
    from concourse.bass_utils import run_bass_kernel_spmd

    x = np.ascontiguousarray(x, dtype=np.float32)
    ctrs = np.asarray(ctrs, dtype=np.float32)
    Wv = np.asarray(Wv, dtype=np.float32)
    Ov = np.ascontiguousarray(Ov, dtype=np.float32)
    s = np.asarray(s, dtype=np.float32)

    # host-side prep of the small routing constants
    c2sT = np.ascontiguousarray((2.0 * ctrs * s[None, :]).T)        # [g, c]
    nccs = np.ascontiguousarray(-((ctrs * ctrs) @ s)[:, None])      # [c, 1]
    wvT = np.ascontiguousarray(Wv.transpose(1, 0, 2).reshape(D, C * P))  # [g, c*p]

    in_maps = []
    for i in range(N_CORES):
        xi = x[i * N_LOC:(i + 1) * N_LOC]
        m = {
            "xT": np.ascontiguousarray(xi.T),
            "wvT": wvT,
            "c2sT": c2sT,
            "nccs": nccs,
            "ov": Ov,
        }
        if REP_NUM > 0:
            m["onesr"] = np.ones((1, P), dtype=np.float32)
        in_maps.append(m)

    nc = _get_nc()
    res = run_bass_kernel_spmd(
        nc, in_maps, core_ids=list(range(N_CORES)), **(_spmd_kwargs or {})
    )
    out = np.empty((N, P), dtype=np.float32)
    for i in range(N_CORES):
        out[i * N_LOC:(i + 1) * N_LOC] = res.results[i]["outT"].T
    kernel.last_result = res
    return out
